# revision 61
# baseline (speedup 1.0000x reference)
"""Trainium2 Bass kernel for nn_DSDModules_57681410785615 (sparse_attention).

Strategy (expert-parallel over groups G=8, one group per NeuronCore, plus
pixel-parallel warp):
  - Each core runs its group's 4-layer 3x3 conv stack as shifted matmuls.
    conv1 (k=512, m=128) and conv2 (k=128x9taps, m=64) run in fp8 e4m3 with
    perf_mode=DoubleRow (contraction packs 2 slices per PE cell -> ~1.44x):
    conv1 pairs the four 128-channel cin chunks; conv2 pairs vertical taps
    (di=0,1) via a hand-built overlapping-stride AP, leaving di=2 as plain
    fp8 matmuls.  Weights are pre-scaled by 512 host-side (fp8 subnormal
    dodge) and unscaled in the fused bias+PReLU evacuation (scalar engine).
    conv3/conv4 stay bf16 with 2-band PE row tiling.
  - conv4 output (18ch = 6 logits + 12 offsets, host-permuted, x64
    pre-scaled) is written per 512-px chunk straight to the AllToAll input
    [8 slices, 18, 512] fp8.  A single fp8 AllToAll hands each core every
    group's logits+offsets for its own 512-pixel slice -- replaces the
    baseline's exp-AllReduce + Wd-AllToAll and their DRAM round trips.
  - Locally per core: softmax across groups via one [96,96] matmul that
    sums over the 8 sources AND broadcasts (then approx reciprocal);
    bilinear axis maps as one fat [96,512] chain covering both axes with
    NO floor/casts: u = off*64/63 + j/63 - 0.5 is the sample displacement,
    per-pixel clamp-bound const tiles reproduce the border clamp, and the
    tap indicator is just (u<0).
  - Wd = sum_k attn*wy*wx per 9 taps via a selection matmul, evacuated and
    repacked per half so the first M matmuls start early; M = mask @ Wd
    rank-8 matmuls into paired 2-bank PSUM tiles, one scalar-engine bf16
    evacuation each; 9-point warp stencil with bf16 DVE multiplies; output
    is the core's own 512-px slice (disjoint; host concatenates).

Self-contained: hardcodes all shapes; no file reads.
"""
import os
import sys
import contextlib

for _p in ('/opt/trn_rl_repo', '/opt/trn_rl_repo/concourse'):
    if _p not in sys.path:
        sys.path.insert(0, _p)

import numpy as np
import ml_dtypes

import concourse.bass as bass
import concourse.mybir as mybir
import concourse.tile as tile
from concourse import bacc
from concourse.ap import AP
from concourse.bass_utils import run_bass_kernel_spmd

BF16 = ml_dtypes.bfloat16
FP8 = ml_dtypes.float8_e4m3
F32 = mybir.dt.float32
BF = mybir.dt.bfloat16
F8 = mybir.dt.float8e4
I32 = mybir.dt.int32

G, K, C_IN, C_FEAT, H, W, B = 8, 6, 512, 256, 64, 64, 1
HW = H * W                  # 4096
PADW = 66                   # conv row width (h2/h3, bf16 convs)
NPAD = PADW * PADW          # 4356
FPAD = 67                   # front guard so tap offsets never go negative
XPL = 4496                  # x/h1 plane stride (67+4356+73, %16==0)
# conv1/conv2 flat-window chunks: (base flat pos, free len, first row, nrows)
# 9 chunks of 7 full padded rows + 1 chunk of row 64; DoubleRow needs a 3D
# moving AP so the window must be contiguous in the flat padded layout.
CCHUNKS = [(66 + 462 * r, 462, 1 + 7 * r, 7) for r in range(9)] + \
          [(66 + 462 * 9, 66, 64, 1)]
SL = 512                    # per-core pixel slice
IMG_W = 66 + SL + 66        # 644: per-core warp image window
S1 = 512.0                  # conv1 weight pre-scale (fp8 subnormal dodge)
S2 = 512.0                  # conv2 weight pre-scale
AluOp = mybir.AluOpType
ActFn = mybir.ActivationFunctionType
PerfM = mybir.MatmulPerfMode

# conv4 output channel permutation: [logit_k (6), offx_k (6), offy_k (6)]
PERM4 = [12 + k for k in range(K)] + [2 * k for k in range(K)] + [2 * k + 1 for k in range(K)]

_CACHE = {}

F_SYNC0 = not os.environ.get("K_NO_SYNC0")   # early dummy collective (skew)
F_GPX = bool(os.environ.get("K_GPX"))        # map products on GpSimd
                                             # (slower per-op; off by default)
F_DBG = bool(os.environ.get("K_DBG"))        # dump intermediates


def _build():
    nc = bacc.Bacc('TRN2', target_bir_lowering=False, debug=False, num_devices=G)

    # ---- inputs (per-core data differs, program identical) ----
    x_dr = nc.dram_tensor("x_dr", [2, 128, 2, XPL], F8, kind="ExternalInput")
    img_f = nc.dram_tensor("img_f", [2, 128, IMG_W], BF, kind="ExternalInput")
    img_s = nc.dram_tensor("img_s", [2, 128, IMG_W], BF, kind="ExternalInput")
    w1d = nc.dram_tensor("w1d", [128, 36, 128], F8, kind="ExternalInput")
    w2d = nc.dram_tensor("w2d", [128, 9, 64], F8, kind="ExternalInput")
    w3d = nc.dram_tensor("w3d", [128, 9, 32], BF, kind="ExternalInput")
    w4d = nc.dram_tensor("w4d", [128, 9, 18], BF, kind="ExternalInput")
    b1d = nc.dram_tensor("b1d", [128, 1], F32, kind="ExternalInput")
    b2d = nc.dram_tensor("b2d", [64, 1], F32, kind="ExternalInput")
    b3d = nc.dram_tensor("b3d", [32, 1], F32, kind="ExternalInput")
    b4d = nc.dram_tensor("b4d", [18, 1], F32, kind="ExternalInput")
    jscd = nc.dram_tensor("jscd", [96, 512], F32, kind="ExternalInput")
    lcd = nc.dram_tensor("lcd", [96, 512], F32, kind="ExternalInput")
    hcd = nc.dram_tensor("hcd", [96, 512], F32, kind="ExternalInput")
    sel16d = nc.dram_tensor("sel16d", [96, 16], BF, kind="ExternalInput")
    selbcd = nc.dram_tensor("selbcd", [96, 96], BF, kind="ExternalInput")
    maskd = nc.dram_tensor("maskd", [8, 256], BF, kind="ExternalInput")

    out_part = nc.dram_tensor("out_part", [2, 128, SL], F32, kind="ExternalOutput")
    if F_DBG:
        dbg_h1 = nc.dram_tensor("dbg_h1", [128, XPL], F8, kind="ExternalOutput")
        dbg_oa = nc.dram_tensor("dbg_oa", [18, HW], F8, kind="ExternalOutput")
        dbg_wd = nc.dram_tensor("dbg_wd", [16, 2304], BF, kind="ExternalOutput")
        dbg_at = nc.dram_tensor("dbg_at", [96, 256], F32, kind="ExternalOutput")
        dbg_lgt = nc.dram_tensor("dbg_lgt", [96, 256], F8, kind="ExternalOutput")
        dbg_oxt = nc.dram_tensor("dbg_oxt", [96, 256], F8, kind="ExternalOutput")
        dbg_w6 = nc.dram_tensor("dbg_w6", [96, 1536], F32, kind="ExternalOutput")
        dbg_prod = nc.dram_tensor("dbg_prod", [96, 2304], BF, kind="ExternalOutput")

    with tile.TileContext(nc) as tc:
        with tc.tile_pool(name="consts", bufs=1) as consts, \
             tc.tile_pool(name="wpool", bufs=1) as wpool, \
             tc.tile_pool(name="hbufs", bufs=1) as hbufs, \
             tc.tile_pool(name="chunks", bufs=3) as chunks, \
             tc.tile_pool(name="dram", bufs=1, space="DRAM") as dram:

            # early dummy collective: absorbs cross-core launch skew while
            # input DMAs + conv1 run, so the real AllToAll doesn't stall.
            if F_SYNC0:
                sync_in = dram.tile([1, 8], F32)
                sync_out = dram.tile([1, 8], F32, addr_space="Shared")
                sync_sb = consts.tile([1, 8], F32)
                nc.vector.memset(sync_sb[:], 0.0)
                nc.sync.dma_start(sync_in[:], sync_sb[:])
                nc.gpsimd.collective_compute(
                    "AllReduce", AluOp.add,
                    replica_groups=[list(range(G))],
                    ins=[sync_in.opt()], outs=[sync_out.opt()])

            # ---- weight/const tiles ----
            w2_t = wpool.tile([128, 9, 64], F8)
            w3_t = wpool.tile([128, 9, 32], BF)
            w4_t = wpool.tile([128, 9, 18], BF)
            b1_t = consts.tile([128, 1], F32)
            b2_t = consts.tile([64, 1], F32)
            b3_t = consts.tile([32, 1], F32)
            b4_t = consts.tile([18, 1], F32)
            jsc = consts.tile([96, 512], F32)
            lc = consts.tile([96, 512], F32)
            hc = consts.tile([96, 512], F32)
            sel16 = consts.tile([96, 16], BF)
            selbc = consts.tile([96, 96], BF)
            maskT = consts.tile([8, 256], BF)
            imf = [consts.tile([128, IMG_W], BF, name=f"imf{c}") for c in range(2)]
            ims = [consts.tile([128, IMG_W], BF, name=f"ims{c}") for c in range(2)]

            def _load_late_consts():
                nc.sync.dma_start(w2_t[:], w2d[:, :, :])
                nc.sync.dma_start(w3_t[:], w3d[:, :, :])
                nc.sync.dma_start(w4_t[:], w4d[:, :, :])
                nc.sync.dma_start(b1_t[:], b1d[:, :])
                nc.sync.dma_start(b2_t[:], b2d[:, :])
                nc.sync.dma_start(b3_t[:], b3d[:, :])
                nc.sync.dma_start(b4_t[:], b4d[:, :])
                nc.sync.dma_start(jsc[:], jscd[:, :])
                nc.sync.dma_start(lc[:], lcd[:, :])
                nc.sync.dma_start(hc[:], hcd[:, :])
                nc.sync.dma_start(sel16[:], sel16d[:, :])
                nc.sync.dma_start(selbc[:], selbcd[:, :])
                nc.sync.dma_start(maskT[:], maskd[:, :])
                for c in range(2):
                    nc.sync.dma_start(imf[c][:], img_f[c, :, :])
                    nc.sync.dma_start(ims[c][:], img_s[c, :, :])

            # hidden activation buffers (padded layout, zeroed borders only)
            h1 = hbufs.tile([128, XPL], F8)
            h2 = hbufs.tile([128, NPAD], BF)
            h3 = hbufs.tile([128, NPAD], BF)
            h1v = h1[:, FPAD:FPAD + NPAD].rearrange("p (r c) -> p r c", c=PADW)
            nc.vector.memset(h1[:, 0:FPAD + PADW], 0.0)
            nc.vector.memset(h1[:, FPAD + 65 * PADW:], 0.0)
            nc.vector.memset(h1v[:, 1:65, 0:1], 0.0)
            nc.vector.memset(h1v[:, 1:65, 65:66], 0.0)
            for hb in (h2, h3):
                v = hb[:].rearrange("p (r c) -> p r c", c=PADW)
                nc.vector.memset(hb[:, 0:PADW], 0.0)
                nc.vector.memset(hb[:, 65 * PADW:], 0.0)
                nc.vector.memset(v[:, 1:65, 0:1], 0.0)
                nc.vector.memset(v[:, 1:65, 65:66], 0.0)
            # conv4 pads its 32-row contraction to 64: the padding rows of h3
            # multiply zero weights, but must not hold NaN bit patterns.
            nc.vector.memset(h3[32:64, :], 0.0)
            nc.vector.memset(h3[96:128, :], 0.0)

            # ---- conv1: fp8 DoubleRow over cin-chunk pairs; contiguous
            # flat 462-windows (7 padded rows; border cols computed and
            # discarded at evac) so the moving AP stays 3D [128, 2, L] ----
            with tc.tile_pool(name="convin", bufs=1) as convin, \
                 tc.tile_pool(name="pc1", bufs=1, space="PSUM") as pc1:
                w1_t = convin.tile([128, 36, 128], F8)
                # 3-way split so the first tap's weights land early
                for wseg in range(3):
                    nc.sync.dma_start(w1_t[:, wseg * 12:(wseg + 1) * 12, :],
                                      w1d[:, wseg * 12:(wseg + 1) * 12, :])
                x_t = [convin.tile([128, 2, XPL], F8, name=f"x{p}") for p in range(2)]
                SPLIT = FPAD + 30 * 66   # covers chunks 0-3 (rows 1..28)
                for p in range(2):
                    nc.sync.dma_start(x_t[p][:, :, 0:SPLIT], x_dr[p, :, :, 0:SPLIT])
                for p in range(2):
                    nc.sync.dma_start(x_t[p][:, :, SPLIT:],
                                      x_dr[p, :, :, SPLIT:])
                _load_late_consts()
                w1v = w1_t[:].rearrange("p (t pr ko) m -> p t pr ko m", pr=2, ko=2)

                for grp in range(3):
                    cks = CCHUNKS[grp * 4:grp * 4 + 4]
                    ps = [pc1.tile([128, 462], F32, tag=f"c1_{c}", bufs=2,
                                   name=f"c1p{c}") for c in range(len(cks))]
                    i_mm = 0
                    for t in range(9):
                        di, dj = t // 3, t % 3
                        toff = (di - 1) * 66 + (dj - 1)
                        for pr in range(2):
                            for c, (base, ln, row0, nr) in enumerate(cks):
                                o = FPAD + base + toff
                                nc.tensor.matmul(ps[c][0:128, 0:ln],
                                                 w1v[:, t, pr, :, :],
                                                 x_t[pr][:, :, o:o + ln],
                                                 start=(i_mm == 0),
                                                 stop=(i_mm == 17),
                                                 perf_mode=PerfM.DoubleRow)
                            i_mm += 1
                    for c, (base, ln, row0, nr) in enumerate(cks):
                        pv = ps[c][0:128, 0:ln].rearrange(
                            "p (r cc) -> p r cc", cc=66)
                        nc.scalar.activation(h1v[:, row0:row0 + nr, 1:65],
                                             pv[:, :, 1:65], ActFn.Prelu,
                                             bias=b1_t[:, 0:1], scale=1.0 / S1,
                                             alpha=0.1)

            # ---- conv2: fp8, vertical tap pairs (di=0,1) DoubleRow (ko
            # step = 66) + di=2 plain; same contiguous windows ----
            h1h = h1[:]
            h2v = h2[:].rearrange("p (r c) -> p r c", c=PADW)
            with tc.tile_pool(name="pc2", bufs=1, space="PSUM") as pc2:
                for grp in range(3):
                    cks = CCHUNKS[grp * 4:grp * 4 + 4]
                    ps2 = [pc2.tile([64, 462], F32, tag=f"c2_{c}", bufs=2,
                                    name=f"c2p{c}") for c in range(len(cks))]
                    i_mm = 0
                    for dj in range(3):
                        for c, (base, ln, row0, nr) in enumerate(cks):
                            o = FPAD + base - 66 + (dj - 1)
                            rhs = AP(h1h.tensor, h1h.offset + o,
                                     [[XPL, 128], [66, 2], [1, ln]])
                            nc.tensor.matmul(ps2[c][0:64, 0:ln],
                                             w2_t[:, 2 * dj:2 * dj + 2, :],
                                             rhs, start=(i_mm == 0), stop=False,
                                             perf_mode=PerfM.DoubleRow)
                        i_mm += 1
                    for dj in range(3):
                        for c, (base, ln, row0, nr) in enumerate(cks):
                            o = FPAD + base + 66 + (dj - 1)
                            nc.tensor.matmul(ps2[c][0:64, 0:ln],
                                             w2_t[:, 6 + dj, :],
                                             h1[:, o:o + ln],
                                             start=False,
                                             stop=(dj == 2))
                        i_mm += 1
                    for c, (base, ln, row0, nr) in enumerate(cks):
                        pv = ps2[c][0:64, 0:ln].rearrange(
                            "p (r cc) -> p r cc", cc=66)
                        nc.scalar.activation(h2v[0:64, row0:row0 + nr, 1:65],
                                             pv[:, :, 1:65], ActFn.Prelu,
                                             bias=b2_t[:, 0:1], scale=1.0 / S2,
                                             alpha=0.1)
                        # band replication pipelined per chunk (incl border
                        # cols of these rows, memset earlier)
                        nc.sync.dma_start(
                            h2[64:128, row0 * PADW:(row0 + nr) * PADW],
                            h2[0:64, row0 * PADW:(row0 + nr) * PADW])
                # rows 0 and 65 of band 1 (borders; never written by evac)
                nc.sync.dma_start(h2[64:128, 0:PADW], h2[0:64, 0:PADW])
                nc.sync.dma_start(h2[64:128, 65 * PADW:], h2[0:64, 65 * PADW:])

            # ---- conv3: k=64 row tiling, 2 bands stream concurrently ----
            h3v = h3[:].rearrange("p (r c) -> p r c", c=PADW)
            with tc.tile_pool(name="pc3", bufs=1, space="PSUM") as pc3:
                for r in range(8):
                    band = r % 2
                    pb = band * 64
                    ps3 = pc3.tile([128, 512], F32, tag=f"c3{band}", bufs=2,
                                   name=f"ps3{band}")
                    for t in range(9):
                        di, dj = t // 3, t % 3
                        nc.tensor.matmul(
                            ps3[0:32, :], w3_t[pb:pb + 64, t, :],
                            h2v[pb:pb + 64, r * 8 + di:r * 8 + di + 8, dj:dj + 64],
                            start=(t == 0), stop=(t == 8))
                    nc.scalar.activation(h3v[0:32, r * 8 + 1:r * 8 + 9, 1:65],
                                         ps3[0:32, :], ActFn.Prelu,
                                         bias=b3_t[:, 0:1], alpha=0.1)
                # replicate h3 into partition band 64:96 for conv4
                nc.sync.dma_start(h3[64:96, :], h3[0:32, :])

            a2a_in = dram.tile([8, 18, 512], F8)
            a2a_out = dram.tile([8, 18, 512], F8)
            oa = hbufs.tile([18, HW], F8)

            # ---- conv4: contraction padded 32->64 (zero weights), 2-band
            # row tiling; each chunk streams straight into the AllToAll in ----
            with tc.tile_pool(name="pc4", bufs=1, space="PSUM") as pc4:
                for r in range(8):
                    band = r % 2
                    pb = band * 64
                    ps4 = pc4.tile([128, 512], F32, tag=f"c4{band}", bufs=2,
                                   name=f"ps4{band}")
                    for t in range(9):
                        di, dj = t // 3, t % 3
                        nc.tensor.matmul(
                            ps4[0:18, :], w4_t[pb:pb + 64, t, :],
                            h3v[pb:pb + 64, r * 8 + di:r * 8 + di + 8, dj:dj + 64],
                            start=(t == 0), stop=(t == 8))
                    # x64 pre-scale dodges fp8 subnormals (b4 host-scaled too)
                    nc.scalar.activation(oa[:, r * 512:(r + 1) * 512], ps4[0:18, :],
                                         ActFn.Identity, bias=b4_t[:, 0:1],
                                         scale=64.0)
                    nc.sync.dma_start(a2a_in[r, :, :],
                                      oa[:, r * 512:(r + 1) * 512])

            # ---- one fp8 AllToAll (collectives here are latency-bound:
            # one 73.7KB op beats two smaller ones) ----
            nc.gpsimd.collective_compute(
                "AllToAll", AluOp.bypass,
                replica_groups=[list(range(G))],
                ins=[a2a_in.opt()], outs=[a2a_out.opt()])
            if F_DBG:
                nc.sync.dma_start(dbg_h1[:, :], h1[:])
                nc.sync.dma_start(dbg_oa[:, :], oa[:])

            _late = contextlib.ExitStack()
            maps = _late.enter_context(tc.tile_pool(name="maps", bufs=1))
            mtmp = _late.enter_context(tc.tile_pool(name="mtmp", bufs=10))

            # partition p = s*12 + k*2 + h  (s=src group, k=tap, h=px half)
            a2av = a2a_out[:].rearrange("s c (hh f) -> s c hh f", f=256)
            lgt = maps.tile([96, 256], F8)
            oxy = maps.tile([96, 512], F8)   # x offsets | y offsets
            # plain 2D dst: DMA streams the 4D DRAM src in flat (s,k,hh,f)
            # order onto partitions.  Different queues so they overlap.
            nc.sync.dma_start(oxy[:, 0:256], a2av[:, 6:12, :, :])
            nc.gpsimd.dma_start(oxy[:, 256:512], a2av[:, 12:18, :, :])
            nc.scalar.dma_start(lgt[:], a2av[:, 0:6, :, :])
            if F_DBG:
                nc.sync.dma_start(dbg_lgt[:, :], lgt[:])
                nc.sync.dma_start(dbg_oxt[:, :], oxy[:, 0:256])

            # softmax across groups: exp -> one [96,96] matmul that both sums
            # over the 8 sources and broadcasts back -> reciprocal -> attn
            ex96 = maps.tile([96, 256], BF)
            nc.scalar.activation(ex96[:], lgt[:], ActFn.Exp, scale=1.0 / 64.0)
            at96 = maps.tile([96, 256], F32)
            with tc.tile_pool(name="psm", bufs=1, space="PSUM") as psm:
                den = psm.tile([96, 256], F32, tag="den", bufs=1, name="den")
                nc.tensor.matmul(den[:], selbc[:], ex96[:], start=True, stop=True)
                rec = mtmp.tile([96, 256], F32, tag="r", name="rec")
                nc.vector.reciprocal_approx_fast(out=rec[:], in_=den[:])
                nc.vector.tensor_tensor(at96[:], ex96[:], rec[:], AluOp.mult)

            # both axes in one fat [96, 512] chain (cols 0:256 = x, 256: = y).
            # u = off*64/63 + j/63 - 0.5 is the sample displacement from its
            # own pixel; per-pixel clamp bounds lc/hc reproduce the border
            # clamp and d0 = -(u<0) -- no floor, no casts.
            # algebraic form: with neg = (uc<0), the three tap weights are
            # wm = -uc*neg, wp = uc*(1-neg) = uc + wm, w0 = 1 - wm - wp
            def mt(nm, w=512):
                return mtmp.tile([96, w], F32, tag="mt", name=nm)
            x = mt("x")
            nc.vector.scalar_tensor_tensor(x[:], oxy[:], 64.0 / 63.0 / 64.0,
                                           jsc[:], AluOp.mult, AluOp.add)
            u1 = mt("u1")
            nc.vector.tensor_tensor(u1[:], x[:], lc[:], AluOp.max)
            uc = mt("uc")
            nc.vector.tensor_tensor(uc[:], u1[:], hc[:], AluOp.min)
            neg = mt("neg")
            nc.vector.tensor_scalar(neg[:], uc[:], 0.0, None, AluOp.is_lt)
            wmf = mt("wmf")
            nc.vector.scalar_tensor_tensor(wmf[:], uc[:], -1.0, neg[:],
                                           AluOp.mult, AluOp.mult)
            wpf = mt("wpf")
            nc.vector.tensor_tensor(wpf[:], uc[:], wmf[:], AluOp.add)
            ssum = mt("ssum")
            nc.vector.tensor_tensor(ssum[:], wmf[:], wpf[:], AluOp.add)
            # bf16 copies of the weights: the 9 prod multiplies then run at
            # the 16-bit DVE rate
            wm = maps.tile([96, 512], BF, name="wm")
            nc.vector.tensor_copy(wm[:], wmf[:])
            wp = maps.tile([96, 512], BF, name="wp")
            nc.vector.tensor_copy(wp[:], wpf[:])
            w0 = maps.tile([96, 512], BF, name="w0")
            nc.vector.tensor_scalar(w0[:], ssum[:], -1.0, 1.0,
                                    AluOp.mult, AluOp.add)
            wxs = {dv: t[:, 0:256] for dv, t in ((-1, wm), (0, w0), (1, wp))}
            wys = {dv: t[:, 256:512] for dv, t in ((-1, wm), (0, w0), (1, wp))}
            if F_DBG:
                nc.sync.dma_start(dbg_at[:, :], at96[:])
                for i, wt in enumerate([wxs[-1], wxs[0], wxs[1]]):
                    nc.sync.dma_start(dbg_w6[:, i * 256:(i + 1) * 256], wt)
                for i, wt in enumerate([wys[-1], wys[0], wys[1]]):
                    nc.sync.dma_start(dbg_w6[:, (3 + i) * 256:(4 + i) * 256],
                                      wt)

            prod = maps.tile([96, 2304], BF)
            for yi, dyv in enumerate((-1, 0, 1)):
                ad = mtmp.tile([96, 256], BF, tag="ad", name="ad")
                nc.vector.tensor_tensor(ad[:], at96[:], wys[dyv], AluOp.mult)
                for xi, dxv in enumerate((-1, 0, 1)):
                    di9 = yi * 3 + xi
                    nc.vector.tensor_tensor(prod[:, di9 * 256:(di9 + 1) * 256],
                                            ad[:], wxs[dxv], AluOp.mult)

            if F_DBG:
                nc.sync.dma_start(dbg_prod[:, :], prod[:])

            # K-sum via selection matmul -> Wd [16=(s,h), 9*256] -> regroup
            # to [8, 2, 2304] with one SBUF DMA
            wjcat = maps.tile([8, 2, 2304], BF, name="wjcat")
            with tc.tile_pool(name="pwd", bufs=1, space="PSUM") as pwd:
                wps = pwd.tile([16, 2304], F32, tag="wdps", bufs=1, name="wdps")
                for c0 in range(0, 2304, 512):
                    cn = min(512, 2304 - c0)
                    nc.tensor.matmul(wps[:, c0:c0 + cn], sel16[:],
                                     prod[:, c0:c0 + cn], start=True, stop=True)
                # evac + repack pipelined per half on separate engines/queues
                # so the first M matmuls (d<4, cols<1024) start early
                wd16 = maps.tile([16, 2304], BF)
                nc.vector.tensor_copy(wd16[:, 0:1152], wps[:, 0:1152])
                nc.gpsimd.dma_start(wjcat[:, :, 0:1152], wd16[:, 0:1152])
                nc.scalar.activation(wd16[:, 1152:2304], wps[:, 1152:2304],
                                     ActFn.Copy)
                nc.scalar.dma_start(wjcat[:, :, 1152:2304], wd16[:, 1152:2304])
                if F_DBG:
                    nc.sync.dma_start(dbg_wd[:, :], wd16[:])

            # ---- M = mask @ Wd (rank-8 bf16 matmuls), warp the local
            # 512-pixel slice ----
            warp = _late.enter_context(tc.tile_pool(name="warp", bufs=1))
            with tc.tile_pool(name="pm", bufs=1, space="PSUM") as pm:
                for t in range(2):
                    prods = []
                    for dp in range(5):
                        nd = 2 if dp < 4 else 1
                        # pair two taps per 2-bank PSUM tile: one matmul per
                        # bank, ONE scalar bf16 evacuation for both
                        psM = pm.tile([128, 1024], F32, tag="m", bufs=3,
                                      name="psM")
                        for q in range(nd):
                            d = dp * 2 + q
                            nc.tensor.matmul(
                                psM[:, q * 512:(q + 1) * 512],
                                maskT[:, t * 128:(t + 1) * 128],
                                wjcat[:, :, d * 256:(d + 1) * 256],
                                start=True, stop=True)
                        psMb = warp.tile([128, 1024], BF, tag=f"mb{dp % 2}",
                                         bufs=2, name=f"psMb{dp % 2}")
                        nc.scalar.activation(psMb[:, 0:nd * 512],
                                             psM[:, 0:nd * 512], ActFn.Copy)
                        for q in range(nd):
                            d = dp * 2 + q
                            dy, dx = d // 3 - 1, d % 3 - 1
                            if dx == 0:
                                img_ap = imf[t][:, 66 + 64 * dy:66 + 64 * dy + SL]
                            elif dx == 1:
                                img_ap = ims[t][:, 66 + 64 * dy:66 + 64 * dy + SL]
                            else:
                                img_ap = ims[t][:, 64 + 64 * dy:64 + 64 * dy + SL]
                            pr = warp.tile([128, 512], BF, tag=f"pr{d}", bufs=2,
                                           name=f"pr{d}")
                            nc.vector.tensor_tensor(pr[:], img_ap,
                                                    psMb[:, q * 512:(q + 1) * 512],
                                                    AluOp.mult)
                            prods.append(pr)
                    # tree sum of 9 products, mostly DVE with GpSimd assist
                    s01 = warp.tile([128, 512], BF, tag="s01", bufs=2, name="s01")
                    nc.vector.tensor_tensor(s01[:], prods[0][:], prods[1][:], AluOp.add)
                    s23 = warp.tile([128, 512], BF, tag="s23", bufs=2, name="s23")
                    nc.vector.tensor_tensor(s23[:], prods[2][:], prods[3][:], AluOp.add)
                    s45 = warp.tile([128, 512], BF, tag="s45", bufs=2, name="s45")
                    nc.vector.tensor_tensor(s45[:], prods[4][:], prods[5][:], AluOp.add)
                    s67 = warp.tile([128, 512], BF, tag="s67", bufs=2, name="s67")
                    nc.gpsimd.tensor_tensor(s67[:], prods[6][:], prods[7][:], AluOp.add)
                    s03 = warp.tile([128, 512], BF, tag="s03", bufs=2, name="s03")
                    nc.vector.tensor_tensor(s03[:], s01[:], s23[:], AluOp.add)
                    s47 = warp.tile([128, 512], BF, tag="s47", bufs=2, name="s47")
                    nc.vector.tensor_tensor(s47[:], s45[:], s67[:], AluOp.add)
                    s07 = warp.tile([128, 512], BF, tag="s07", bufs=2, name="s07")
                    nc.vector.tensor_tensor(s07[:], s03[:], s47[:], AluOp.add)
                    out_t = warp.tile([128, 512], F32, tag="out", bufs=2, name="out_t")
                    nc.vector.tensor_tensor(out_t[:], s07[:], prods[8][:], AluOp.add)
                    nc.sync.dma_start(out_part[t, :, :], out_t[:])
            _late.close()

    nc.compile()
    return nc


def _prep_inputs(gar_feat, cond_feat, mask, W1, b1, W2, b2, W3, b3, W4, b4):
    """Host-side prep: returns list of 8 per-core input dicts."""
    gar = np.asarray(gar_feat, np.float32)[0]      # [256, 64, 64]
    cond = np.asarray(cond_feat, np.float32)[0]
    maskf = np.asarray(mask, np.float32)[0]        # [G, 256]

    inp = np.concatenate([gar, cond], axis=0)      # [512, 64, 64]
    inp_pad = np.zeros((C_IN, PADW, PADW), np.float32)
    inp_pad[:, 1:-1, 1:-1] = inp
    inp_chunks = inp_pad.reshape(4, 128, NPAD)
    x_np = np.zeros((2, 128, 2, XPL), np.float32)
    for pr in range(2):
        for ko in range(2):
            x_np[pr, :, ko, FPAD:FPAD + NPAD] = inp_chunks[pr * 2 + ko]
    x_np = np.clip(x_np, -240, 240).astype(FP8)

    # per-slice pixel coords (partition p = s*12 + k*2 + h; rows identical)
    # px_global = g*512 + h*256 + f
    def coords(g):
        pj = np.zeros((2, 256), np.float32)
        pi = np.zeros((2, 256), np.float32)
        for h in range(2):
            gpx = g * 512 + h * 256 + np.arange(256)
            pi[h] = gpx // W
            pj[h] = gpx % W
        i96 = np.tile(pi, (48, 1)).astype(np.float32)
        j96 = np.tile(pj, (48, 1)).astype(np.float32)
        return i96, j96

    sel16 = np.zeros((96, 16), np.float32)
    selbc = np.zeros((96, 96), np.float32)
    for p in range(96):
        s, rem = divmod(p, 12)
        k, h = divmod(rem, 2)
        sel16[p, s * 2 + h] = 1.0
        for s2 in range(8):
            selbc[s2 * 12 + rem, p] = 1.0
    sel16 = sel16.astype(BF16)
    selbc = selbc.astype(BF16)
    maskT = maskf.astype(BF16)                     # [8, 256]

    # flat gar image with wide guard, plus shift-by-one copy (for odd bases)
    gar_flat = gar.reshape(2, 128, HW)
    gpad = np.zeros((2, 128, 66 + HW + 67), np.float32)
    gpad[:, :, 66:66 + HW] = gar_flat

    per_core = []
    for g in range(G):
        w1g = np.asarray(W1[g], np.float32)   # [128, 512, 3, 3]
        w2g = np.asarray(W2[g], np.float32)   # [64, 128, 3, 3]
        w3g = np.asarray(W3[g], np.float32)   # [32, 64, 3, 3]
        w4g = np.asarray(W4[g], np.float32)[PERM4]   # [18, 32, 3, 3] permuted
        b4g = np.asarray(b4[g], np.float32)[PERM4]

        w1T = np.zeros((128, 36, 128), np.float32)
        for t in range(9):
            di, dj = t // 3, t % 3
            for pr in range(2):
                for ko in range(2):
                    idx = (t * 2 + pr) * 2 + ko
                    cin = (pr * 2 + ko) * 128
                    w1T[:, idx, :] = w1g[:, cin:cin + 128, di, dj].T * S1
        w1T = np.clip(w1T, -240, 240).astype(FP8)

        w2T = np.zeros((128, 9, 64), np.float32)
        for dj in range(3):
            for ko in range(2):  # taps (di=0,1) as DoubleRow pairs
                w2T[:, 2 * dj + ko, :] = w2g[:, :, ko, dj].T * S2
            w2T[:, 6 + dj, :] = w2g[:, :, 2, dj].T * S2
        w2T = np.clip(w2T, -240, 240).astype(FP8)

        w3T = np.zeros((128, 9, 32), np.float32)
        w4T = np.zeros((128, 9, 18), np.float32)
        for t in range(9):
            di, dj = t // 3, t % 3
            for bnd in range(2):
                w3T[bnd * 64:(bnd + 1) * 64, t, :] = w3g[:, :, di, dj].T
                # conv4 contraction is padded to 64 rows; rows 32:64 / 96:128
                # stay zero so the padding contributes nothing.
                w4T[bnd * 64:bnd * 64 + 32, t, :] = w4g[:, :, di, dj].T

        # per-core warp windows: global pixels [g*512-66, g*512+512+66)
        base = g * SL
        imgf = gpad[:, :, base:base + IMG_W]                  # offset -66
        imgs = gpad[:, :, base + 1:base + 1 + IMG_W]          # shift +1

        i96, j96 = coords(g)
        jsc = np.concatenate([j96 / 63.0 - 0.5, i96 / 63.0 - 0.5],
                             axis=1).astype(np.float32)
        lc = np.concatenate([np.maximum(-j96, -1.0), np.maximum(-i96, -1.0)],
                            axis=1).astype(np.float32)
        hc = np.concatenate([np.minimum(62.999 - j96, 1.0),
                             np.minimum(62.999 - i96, 1.0)],
                            axis=1).astype(np.float32)

        per_core.append({
            "x_dr": x_np,
            "img_f": np.ascontiguousarray(imgf).astype(BF16),
            "img_s": np.ascontiguousarray(imgs).astype(BF16),
            "w1d": w1T,
            "w2d": w2T,
            "w3d": w3T.astype(BF16),
            "w4d": w4T.astype(BF16),
            "b1d": np.asarray(b1[g], np.float32).reshape(128, 1),
            "b2d": np.asarray(b2[g], np.float32).reshape(64, 1),
            "b3d": np.asarray(b3[g], np.float32).reshape(32, 1),
            "b4d": (b4g * 64.0).reshape(18, 1),
            "jscd": jsc, "lcd": lc, "hcd": hc,
            "sel16d": sel16, "selbcd": selbc, "maskd": maskT,
        })
    return per_core


def _get_nc():
    if "nc" not in _CACHE:
        _CACHE["nc"] = _build()
    return _CACHE["nc"]


def run_cores(inputs, trace=False):
    nc = _get_nc()
    in_maps = _prep_inputs(**inputs)
    res = run_bass_kernel_spmd(nc, in_maps, core_ids=list(range(G)), trace=trace)
    return res


def kernel(**inputs) -> np.ndarray:
    res = run_cores(inputs, trace=False)
    out = np.zeros((C_FEAT, HW), np.float32)
    for g, r in enumerate(res.results):
        out[:, g * SL:(g + 1) * SL] = r["out_part"].reshape(C_FEAT, SL)
    return out.reshape(1, C_FEAT, H, W)


# revision 62
# speedup vs baseline: 1.0956x; 1.0956x over previous
"""Trainium2 Bass kernel for nn_DSDModules_57681410785615 (sparse_attention).

Strategy (expert-parallel over groups G=8, one group per NeuronCore, plus
pixel-parallel warp):
  - Each core runs its group's 4-layer 3x3 conv stack as shifted matmuls.
    conv1 (k=512, m=128) and conv2 (k=128x9taps, m=64) run in fp8 e4m3 with
    perf_mode=DoubleRow (contraction packs 2 slices per PE cell -> ~1.44x):
    conv1 pairs the four 128-channel cin chunks; conv2 pairs vertical taps
    (di=0,1) via a hand-built overlapping-stride AP, leaving di=2 as plain
    fp8 matmuls.  Weights are pre-scaled by 512 host-side (fp8 subnormal
    dodge) and unscaled in the fused bias+PReLU evacuation (scalar engine).
    conv3/conv4 stay bf16 with 2-band PE row tiling.
  - conv4 output (18ch = 6 logits + 12 offsets, host-permuted, x64
    pre-scaled) is written per 512-px chunk straight to the AllToAll input
    [8 slices, 18, 512] fp8.  A single fp8 AllToAll hands each core every
    group's logits+offsets for its own 512-pixel slice -- replaces the
    baseline's exp-AllReduce + Wd-AllToAll and their DRAM round trips.
  - Locally per core: softmax across groups via one [96,96] matmul that
    sums over the 8 sources AND broadcasts (then approx reciprocal);
    bilinear axis maps as one fat [96,512] chain covering both axes with
    NO floor/casts: u = off*64/63 + j/63 - 0.5 is the sample displacement,
    per-pixel clamp-bound const tiles reproduce the border clamp, and the
    tap indicator is just (u<0).
  - Wd = sum_k attn*wy*wx per 9 taps via a selection matmul, evacuated and
    repacked per half so the first M matmuls start early; M = mask @ Wd
    rank-8 matmuls into paired 2-bank PSUM tiles, one scalar-engine bf16
    evacuation each; 9-point warp stencil with bf16 DVE multiplies; output
    is the core's own 512-px slice (disjoint; host concatenates).

Self-contained: hardcodes all shapes; no file reads.
"""
import os
import sys
import contextlib

for _p in ('/opt/trn_rl_repo', '/opt/trn_rl_repo/concourse'):
    if _p not in sys.path:
        sys.path.insert(0, _p)

import numpy as np
import ml_dtypes

import concourse.bass as bass
import concourse.mybir as mybir
import concourse.tile as tile
from concourse import bacc
from concourse.ap import AP
from concourse.bass_utils import run_bass_kernel_spmd

BF16 = ml_dtypes.bfloat16
FP8 = ml_dtypes.float8_e4m3
F32 = mybir.dt.float32
BF = mybir.dt.bfloat16
F8 = mybir.dt.float8e4
I32 = mybir.dt.int32

G, K, C_IN, C_FEAT, H, W, B = 8, 6, 512, 256, 64, 64, 1
HW = H * W                  # 4096
PADW = 66                   # conv row width (h2/h3, bf16 convs)
NPAD = PADW * PADW          # 4356
FPAD = 67                   # front guard so tap offsets never go negative
XPL = 4496                  # x/h1 plane stride (67+4356+73, %16==0)
# conv1/conv2 flat-window chunks: (base flat pos, free len, first row, nrows)
# 9 chunks of 7 full padded rows + 1 chunk of row 64; DoubleRow needs a 3D
# moving AP so the window must be contiguous in the flat padded layout.
CCHUNKS = [(66 + 462 * r, 462, 1 + 7 * r, 7) for r in range(9)] + \
          [(66 + 462 * 9, 66, 64, 1)]
SL = 512                    # per-core pixel slice
IMG_W = 66 + SL + 66        # 644: per-core warp image window
S1 = 512.0                  # conv1 weight pre-scale (fp8 subnormal dodge)
S2 = 512.0                  # conv2 weight pre-scale
AluOp = mybir.AluOpType
ActFn = mybir.ActivationFunctionType
PerfM = mybir.MatmulPerfMode

# conv4 output channel permutation: [logit_k (6), offx_k (6), offy_k (6)]
PERM4 = [12 + k for k in range(K)] + [2 * k for k in range(K)] + [2 * k + 1 for k in range(K)]

_CACHE = {}

F_SYNC0 = not os.environ.get("K_NO_SYNC0")   # early dummy collective (skew)
F_GPX = bool(os.environ.get("K_GPX"))        # map products on GpSimd
                                             # (slower per-op; off by default)
F_DBG = bool(os.environ.get("K_DBG"))        # dump intermediates


def _build():
    nc = bacc.Bacc('TRN2', target_bir_lowering=False, debug=False, num_devices=G)

    # ---- inputs (per-core data differs, program identical) ----
    x_dr = nc.dram_tensor("x_dr", [2, 128, 2, XPL], F8, kind="ExternalInput")
    img_f = nc.dram_tensor("img_f", [2, 128, IMG_W], BF, kind="ExternalInput")
    img_s = nc.dram_tensor("img_s", [2, 128, IMG_W], BF, kind="ExternalInput")
    w1d = nc.dram_tensor("w1d", [128, 36, 128], F8, kind="ExternalInput")
    w2d = nc.dram_tensor("w2d", [128, 9, 64], F8, kind="ExternalInput")
    w3d = nc.dram_tensor("w3d", [128, 9, 32], BF, kind="ExternalInput")
    w4d = nc.dram_tensor("w4d", [128, 9, 18], BF, kind="ExternalInput")
    b1d = nc.dram_tensor("b1d", [128, 1], F32, kind="ExternalInput")
    b2d = nc.dram_tensor("b2d", [64, 1], F32, kind="ExternalInput")
    b3d = nc.dram_tensor("b3d", [32, 1], F32, kind="ExternalInput")
    b4d = nc.dram_tensor("b4d", [18, 1], F32, kind="ExternalInput")
    jscd = nc.dram_tensor("jscd", [96, 512], F32, kind="ExternalInput")
    lcd = nc.dram_tensor("lcd", [96, 512], F32, kind="ExternalInput")
    hcd = nc.dram_tensor("hcd", [96, 512], F32, kind="ExternalInput")
    sel16d = nc.dram_tensor("sel16d", [96, 16], BF, kind="ExternalInput")
    selbcd = nc.dram_tensor("selbcd", [96, 96], BF, kind="ExternalInput")
    maskd = nc.dram_tensor("maskd", [8, 256], BF, kind="ExternalInput")

    out_part = nc.dram_tensor("out_part", [2, 128, SL], F32, kind="ExternalOutput")
    if F_DBG:
        dbg_h1 = nc.dram_tensor("dbg_h1", [128, XPL], F8, kind="ExternalOutput")
        dbg_oa = nc.dram_tensor("dbg_oa", [18, HW], F8, kind="ExternalOutput")
        dbg_wd = nc.dram_tensor("dbg_wd", [16, 2304], BF, kind="ExternalOutput")
        dbg_at = nc.dram_tensor("dbg_at", [96, 256], F32, kind="ExternalOutput")
        dbg_lgt = nc.dram_tensor("dbg_lgt", [96, 256], F8, kind="ExternalOutput")
        dbg_oxt = nc.dram_tensor("dbg_oxt", [96, 256], F8, kind="ExternalOutput")
        dbg_w6 = nc.dram_tensor("dbg_w6", [96, 1536], F32, kind="ExternalOutput")
        dbg_prod = nc.dram_tensor("dbg_prod", [96, 2304], BF, kind="ExternalOutput")

    with tile.TileContext(nc) as tc:
        with tc.tile_pool(name="consts", bufs=1) as consts, \
             tc.tile_pool(name="wpool", bufs=1) as wpool, \
             tc.tile_pool(name="hbufs", bufs=1) as hbufs, \
             tc.tile_pool(name="chunks", bufs=3) as chunks, \
             tc.tile_pool(name="dram", bufs=1, space="DRAM") as dram:

            # early dummy collective: absorbs cross-core launch skew while
            # input DMAs + conv1 run, so the real AllToAll doesn't stall.
            if F_SYNC0:
                sync_in = dram.tile([1, 8], F32)
                sync_out = dram.tile([1, 8], F32, addr_space="Shared")
                sync_sb = consts.tile([1, 8], F32)
                nc.vector.memset(sync_sb[:], 0.0)
                nc.sync.dma_start(sync_in[:], sync_sb[:])
                nc.gpsimd.collective_compute(
                    "AllReduce", AluOp.add,
                    replica_groups=[list(range(G))],
                    ins=[sync_in.opt()], outs=[sync_out.opt()])

            # ---- weight/const tiles ----
            w2_t = wpool.tile([128, 9, 64], F8)
            w3_t = wpool.tile([128, 9, 32], BF)
            w4_t = wpool.tile([128, 9, 18], BF)
            b1_t = consts.tile([128, 1], F32)
            b2_t = consts.tile([64, 1], F32)
            b3_t = consts.tile([32, 1], F32)
            b4_t = consts.tile([18, 1], F32)
            jsc = consts.tile([96, 512], F32)
            lc = consts.tile([96, 512], F32)
            hc = consts.tile([96, 512], F32)
            sel16 = consts.tile([96, 16], BF)
            selbc = consts.tile([96, 96], BF)
            maskT = consts.tile([8, 256], BF)
            imf = [consts.tile([128, IMG_W], BF, name=f"imf{c}") for c in range(2)]
            ims = [consts.tile([128, IMG_W], BF, name=f"ims{c}") for c in range(2)]

            def _load_late_consts():
                nc.sync.dma_start(w2_t[:], w2d[:, :, :])
                nc.sync.dma_start(w3_t[:], w3d[:, :, :])
                nc.sync.dma_start(w4_t[:], w4d[:, :, :])
                nc.sync.dma_start(b1_t[:], b1d[:, :])
                nc.sync.dma_start(b2_t[:], b2d[:, :])
                nc.sync.dma_start(b3_t[:], b3d[:, :])
                nc.sync.dma_start(b4_t[:], b4d[:, :])
                nc.sync.dma_start(jsc[:], jscd[:, :])
                nc.sync.dma_start(lc[:], lcd[:, :])
                nc.sync.dma_start(hc[:], hcd[:, :])
                nc.sync.dma_start(sel16[:], sel16d[:, :])
                nc.sync.dma_start(selbc[:], selbcd[:, :])
                nc.sync.dma_start(maskT[:], maskd[:, :])
                for c in range(2):
                    nc.sync.dma_start(imf[c][:], img_f[c, :, :])
                    nc.sync.dma_start(ims[c][:], img_s[c, :, :])

            # hidden activation buffers (padded layout, zeroed borders only)
            h1 = hbufs.tile([128, XPL], F8)
            h2 = hbufs.tile([128, NPAD], BF)
            h3 = hbufs.tile([128, NPAD], BF)
            h1v = h1[:, FPAD:FPAD + NPAD].rearrange("p (r c) -> p r c", c=PADW)
            nc.vector.memset(h1[:, 0:FPAD + PADW], 0.0)
            nc.vector.memset(h1[:, FPAD + 65 * PADW:], 0.0)
            nc.vector.memset(h1v[:, 1:65, 0:1], 0.0)
            nc.vector.memset(h1v[:, 1:65, 65:66], 0.0)
            for hb in (h2, h3):
                v = hb[:].rearrange("p (r c) -> p r c", c=PADW)
                nc.vector.memset(hb[:, 0:PADW], 0.0)
                nc.vector.memset(hb[:, 65 * PADW:], 0.0)
                nc.vector.memset(v[:, 1:65, 0:1], 0.0)
                nc.vector.memset(v[:, 1:65, 65:66], 0.0)
            # conv4 pads its 32-row contraction to 64: the padding rows of h3
            # multiply zero weights, but must not hold NaN bit patterns.
            nc.vector.memset(h3[32:64, :], 0.0)
            nc.vector.memset(h3[96:128, :], 0.0)

            # ---- conv1: fp8 DoubleRow over cin-chunk pairs; contiguous
            # flat 462-windows (7 padded rows; border cols computed and
            # discarded at evac) so the moving AP stays 3D [128, 2, L] ----
            with tc.tile_pool(name="convin", bufs=1) as convin, \
                 tc.tile_pool(name="pc1", bufs=1, space="PSUM") as pc1:
                w1_t = convin.tile([128, 36, 128], F8)
                # 3-way split so the first tap's weights land early
                for wseg in range(3):
                    nc.sync.dma_start(w1_t[:, wseg * 12:(wseg + 1) * 12, :],
                                      w1d[:, wseg * 12:(wseg + 1) * 12, :])
                x_t = [convin.tile([128, 2, XPL], F8, name=f"x{p}") for p in range(2)]
                SPLIT = FPAD + 30 * 66   # covers chunks 0-3 (rows 1..28)
                for p in range(2):
                    nc.sync.dma_start(x_t[p][:, :, 0:SPLIT], x_dr[p, :, :, 0:SPLIT])
                for p in range(2):
                    nc.sync.dma_start(x_t[p][:, :, SPLIT:],
                                      x_dr[p, :, :, SPLIT:])
                _load_late_consts()
                w1v = w1_t[:].rearrange("p (t pr ko) m -> p t pr ko m", pr=2, ko=2)

                for grp in range(3):
                    cks = CCHUNKS[grp * 4:grp * 4 + 4]
                    ps = [pc1.tile([128, 462], F32, tag=f"c1_{c}", bufs=2,
                                   name=f"c1p{c}") for c in range(len(cks))]
                    i_mm = 0
                    for t in range(9):
                        di, dj = t // 3, t % 3
                        toff = (di - 1) * 66 + (dj - 1)
                        for pr in range(2):
                            for c, (base, ln, row0, nr) in enumerate(cks):
                                o = FPAD + base + toff
                                nc.tensor.matmul(ps[c][0:128, 0:ln],
                                                 w1v[:, t, pr, :, :],
                                                 x_t[pr][:, :, o:o + ln],
                                                 start=(i_mm == 0),
                                                 stop=(i_mm == 17),
                                                 perf_mode=PerfM.DoubleRow)
                            i_mm += 1
                    for c, (base, ln, row0, nr) in enumerate(cks):
                        pv = ps[c][0:128, 0:ln].rearrange(
                            "p (r cc) -> p r cc", cc=66)
                        nc.scalar.activation(h1v[:, row0:row0 + nr, 1:65],
                                             pv[:, :, 1:65], ActFn.Prelu,
                                             bias=b1_t[:, 0:1], scale=1.0 / S1,
                                             alpha=0.1)

            # ---- conv2: fp8, vertical tap pairs (di=0,1) DoubleRow (ko
            # step = 66) + di=2 plain; same contiguous windows ----
            h1h = h1[:]
            h2v = h2[:].rearrange("p (r c) -> p r c", c=PADW)
            with tc.tile_pool(name="pc2", bufs=1, space="PSUM") as pc2:
                for grp in range(3):
                    cks = CCHUNKS[grp * 4:grp * 4 + 4]
                    ps2 = [pc2.tile([64, 462], F32, tag=f"c2_{c}", bufs=2,
                                    name=f"c2p{c}") for c in range(len(cks))]
                    i_mm = 0
                    for dj in range(3):
                        for c, (base, ln, row0, nr) in enumerate(cks):
                            o = FPAD + base - 66 + (dj - 1)
                            rhs = AP(h1h.tensor, h1h.offset + o,
                                     [[XPL, 128], [66, 2], [1, ln]])
                            nc.tensor.matmul(ps2[c][0:64, 0:ln],
                                             w2_t[:, 2 * dj:2 * dj + 2, :],
                                             rhs, start=(i_mm == 0), stop=False,
                                             perf_mode=PerfM.DoubleRow)
                        i_mm += 1
                    for dj in range(3):
                        for c, (base, ln, row0, nr) in enumerate(cks):
                            o = FPAD + base + 66 + (dj - 1)
                            nc.tensor.matmul(ps2[c][0:64, 0:ln],
                                             w2_t[:, 6 + dj, :],
                                             h1[:, o:o + ln],
                                             start=False,
                                             stop=(dj == 2))
                        i_mm += 1
                    for c, (base, ln, row0, nr) in enumerate(cks):
                        pv = ps2[c][0:64, 0:ln].rearrange(
                            "p (r cc) -> p r cc", cc=66)
                        nc.scalar.activation(h2v[0:64, row0:row0 + nr, 1:65],
                                             pv[:, :, 1:65], ActFn.Prelu,
                                             bias=b2_t[:, 0:1], scale=1.0 / S2,
                                             alpha=0.1)
                        # band replication pipelined per chunk (incl border
                        # cols of these rows, memset earlier)
                        nc.sync.dma_start(
                            h2[64:128, row0 * PADW:(row0 + nr) * PADW],
                            h2[0:64, row0 * PADW:(row0 + nr) * PADW])
                # rows 0 and 65 of band 1 (borders; never written by evac)
                nc.sync.dma_start(h2[64:128, 0:PADW], h2[0:64, 0:PADW])
                nc.sync.dma_start(h2[64:128, 65 * PADW:], h2[0:64, 65 * PADW:])

            # ---- conv3: k=64 row tiling, 2 bands stream concurrently ----
            h3v = h3[:].rearrange("p (r c) -> p r c", c=PADW)
            with tc.tile_pool(name="pc3", bufs=1, space="PSUM") as pc3:
                for r in range(8):
                    band = r % 2
                    pb = band * 64
                    ps3 = pc3.tile([128, 512], F32, tag=f"c3{band}", bufs=2,
                                   name=f"ps3{band}")
                    for t in range(9):
                        di, dj = t // 3, t % 3
                        nc.tensor.matmul(
                            ps3[0:32, :], w3_t[pb:pb + 64, t, :],
                            h2v[pb:pb + 64, r * 8 + di:r * 8 + di + 8, dj:dj + 64],
                            start=(t == 0), stop=(t == 8))
                    nc.scalar.activation(h3v[0:32, r * 8 + 1:r * 8 + 9, 1:65],
                                         ps3[0:32, :], ActFn.Prelu,
                                         bias=b3_t[:, 0:1], alpha=0.1)
                # replicate h3 into partition band 64:96 for conv4
                nc.sync.dma_start(h3[64:96, :], h3[0:32, :])

            a2a_in = dram.tile([8, 18, 512], F8)
            a2a_out = dram.tile([8, 18, 512], F8)
            oa = hbufs.tile([18, HW], F8)

            # ---- conv4: contraction padded 32->64 (zero weights), 2-band
            # row tiling; each chunk streams straight into the AllToAll in ----
            with tc.tile_pool(name="pc4", bufs=1, space="PSUM") as pc4:
                for r in range(8):
                    band = r % 2
                    pb = band * 64
                    ps4 = pc4.tile([128, 512], F32, tag=f"c4{band}", bufs=2,
                                   name=f"ps4{band}")
                    for t in range(9):
                        di, dj = t // 3, t % 3
                        nc.tensor.matmul(
                            ps4[0:18, :], w4_t[pb:pb + 64, t, :],
                            h3v[pb:pb + 64, r * 8 + di:r * 8 + di + 8, dj:dj + 64],
                            start=(t == 0), stop=(t == 8))
                    # x64 pre-scale dodges fp8 subnormals (b4 host-scaled too)
                    nc.scalar.activation(oa[:, r * 512:(r + 1) * 512], ps4[0:18, :],
                                         ActFn.Identity, bias=b4_t[:, 0:1],
                                         scale=64.0)
                    nc.sync.dma_start(a2a_in[r, :, :],
                                      oa[:, r * 512:(r + 1) * 512])

            # ---- one fp8 AllToAll (collectives here are latency-bound:
            # one 73.7KB op beats two smaller ones) ----
            nc.gpsimd.collective_compute(
                "AllToAll", AluOp.bypass,
                replica_groups=[list(range(G))],
                ins=[a2a_in.opt()], outs=[a2a_out.opt()])
            if F_DBG:
                nc.sync.dma_start(dbg_h1[:, :], h1[:])
                nc.sync.dma_start(dbg_oa[:, :], oa[:])

            _late = contextlib.ExitStack()
            maps = _late.enter_context(tc.tile_pool(name="maps", bufs=1))
            mtmp = _late.enter_context(tc.tile_pool(name="mtmp", bufs=10))

            # partition p = s*12 + k*2 + h  (s=src group, k=tap, h=px half)
            a2av = a2a_out[:].rearrange("s c (hh f) -> s c hh f", f=256)
            lgt = maps.tile([96, 256], F8)
            oxy = maps.tile([96, 512], F8)   # x offsets | y offsets
            # plain 2D dst: DMA streams the 4D DRAM src in flat (s,k,hh,f)
            # order onto partitions.  Different queues so they overlap.
            nc.sync.dma_start(oxy[:, 0:256], a2av[:, 6:12, :, :])
            nc.gpsimd.dma_start(oxy[:, 256:512], a2av[:, 12:18, :, :])
            nc.scalar.dma_start(lgt[:], a2av[:, 0:6, :, :])
            if F_DBG:
                nc.sync.dma_start(dbg_lgt[:, :], lgt[:])
                nc.sync.dma_start(dbg_oxt[:, :], oxy[:, 0:256])

            # softmax across groups: exp -> one [96,96] matmul that both sums
            # over the 8 sources and broadcasts back -> reciprocal -> attn
            ex96 = maps.tile([96, 256], BF)
            nc.scalar.activation(ex96[:], lgt[:], ActFn.Exp, scale=1.0 / 64.0)
            at96 = maps.tile([96, 256], F32)
            with tc.tile_pool(name="psm", bufs=1, space="PSUM") as psm:
                den = psm.tile([96, 256], F32, tag="den", bufs=1, name="den")
                nc.tensor.matmul(den[:], selbc[:], ex96[:], start=True, stop=True)
                rec = mtmp.tile([96, 256], F32, tag="r", name="rec")
                nc.vector.reciprocal_approx_fast(out=rec[:], in_=den[:])
                nc.vector.tensor_tensor(at96[:], ex96[:], rec[:], AluOp.mult)

            # both axes in one fat [96, 512] chain (cols 0:256 = x, 256: = y).
            # u = off*64/63 + j/63 - 0.5 is the sample displacement from its
            # own pixel; per-pixel clamp bounds lc/hc reproduce the border
            # clamp and d0 = -(u<0) -- no floor, no casts.
            # algebraic form: with neg = (uc<0), the three tap weights are
            # wm = -uc*neg, wp = uc*(1-neg) = uc + wm, w0 = 1 - wm - wp
            def mt(nm, w=512):
                return mtmp.tile([96, w], F32, tag="mt", name=nm)
            x = mt("x")
            nc.vector.scalar_tensor_tensor(x[:], oxy[:], 64.0 / 63.0 / 64.0,
                                           jsc[:], AluOp.mult, AluOp.add)
            u1 = mt("u1")
            nc.vector.tensor_tensor(u1[:], x[:], lc[:], AluOp.max)
            uc = mt("uc")
            nc.vector.tensor_tensor(uc[:], u1[:], hc[:], AluOp.min)
            neg = mt("neg")
            nc.vector.tensor_scalar(neg[:], uc[:], 0.0, None, AluOp.is_lt)
            wm = maps.tile([96, 512], F32, name="wm")
            nc.vector.scalar_tensor_tensor(wm[:], uc[:], -1.0, neg[:],
                                           AluOp.mult, AluOp.mult)
            wp = maps.tile([96, 512], F32, name="wp")
            nc.vector.tensor_tensor(wp[:], uc[:], wm[:], AluOp.add)
            ssum = mt("ssum")
            nc.vector.tensor_tensor(ssum[:], wm[:], wp[:], AluOp.add)
            w0 = maps.tile([96, 512], F32, name="w0")
            nc.vector.tensor_scalar(w0[:], ssum[:], -1.0, 1.0,
                                    AluOp.mult, AluOp.add)
            wxs = {dv: t[:, 0:256] for dv, t in ((-1, wm), (0, w0), (1, wp))}
            wys = {dv: t[:, 256:512] for dv, t in ((-1, wm), (0, w0), (1, wp))}
            if F_DBG:
                nc.sync.dma_start(dbg_at[:, :], at96[:])
                for i, wt in enumerate([wxs[-1], wxs[0], wxs[1]]):
                    nc.sync.dma_start(dbg_w6[:, i * 256:(i + 1) * 256], wt)
                for i, wt in enumerate([wys[-1], wys[0], wys[1]]):
                    nc.sync.dma_start(dbg_w6[:, (3 + i) * 256:(4 + i) * 256],
                                      wt)

            prod = maps.tile([96, 2304], BF)
            for yi, dyv in enumerate((-1, 0, 1)):
                ad = mtmp.tile([96, 256], F32, tag="ad", name="ad")
                nc.vector.tensor_tensor(ad[:], at96[:], wys[dyv], AluOp.mult)
                for xi, dxv in enumerate((-1, 0, 1)):
                    di9 = yi * 3 + xi
                    nc.vector.tensor_tensor(prod[:, di9 * 256:(di9 + 1) * 256],
                                            ad[:], wxs[dxv], AluOp.mult)

            if F_DBG:
                nc.sync.dma_start(dbg_prod[:, :], prod[:])

            # K-sum via selection matmul -> Wd [16=(s,h), 9*256] -> regroup
            # to [8, 2, 2304] with one SBUF DMA
            wjcat = maps.tile([8, 2, 2304], BF, name="wjcat")
            with tc.tile_pool(name="pwd", bufs=1, space="PSUM") as pwd:
                wps = pwd.tile([16, 2304], F32, tag="wdps", bufs=1, name="wdps")
                for c0 in range(0, 2304, 512):
                    cn = min(512, 2304 - c0)
                    nc.tensor.matmul(wps[:, c0:c0 + cn], sel16[:],
                                     prod[:, c0:c0 + cn], start=True, stop=True)
                # evac + repack pipelined per half on separate engines/queues
                # so the first M matmuls (d<4, cols<1024) start early
                wd16 = maps.tile([16, 2304], BF)
                nc.vector.tensor_copy(wd16[:, 0:1152], wps[:, 0:1152])
                nc.gpsimd.dma_start(wjcat[:, :, 0:1152], wd16[:, 0:1152])
                nc.scalar.activation(wd16[:, 1152:2304], wps[:, 1152:2304],
                                     ActFn.Copy)
                nc.scalar.dma_start(wjcat[:, :, 1152:2304], wd16[:, 1152:2304])
                if F_DBG:
                    nc.sync.dma_start(dbg_wd[:, :], wd16[:])

            # ---- M = mask @ Wd (rank-8 bf16 matmuls), warp the local
            # 512-pixel slice ----
            warp = _late.enter_context(tc.tile_pool(name="warp", bufs=1))
            with tc.tile_pool(name="pm", bufs=1, space="PSUM") as pm:
                for t in range(2):
                    prods = []
                    for dp in range(5):
                        nd = 2 if dp < 4 else 1
                        # pair two taps per 2-bank PSUM tile: one matmul per
                        # bank, ONE scalar bf16 evacuation for both
                        psM = pm.tile([128, 1024], F32, tag="m", bufs=3,
                                      name="psM")
                        for q in range(nd):
                            d = dp * 2 + q
                            nc.tensor.matmul(
                                psM[:, q * 512:(q + 1) * 512],
                                maskT[:, t * 128:(t + 1) * 128],
                                wjcat[:, :, d * 256:(d + 1) * 256],
                                start=True, stop=True)
                        psMb = warp.tile([128, 1024], BF, tag=f"mb{dp % 2}",
                                         bufs=2, name=f"psMb{dp % 2}")
                        nc.scalar.activation(psMb[:, 0:nd * 512],
                                             psM[:, 0:nd * 512], ActFn.Copy)
                        for q in range(nd):
                            d = dp * 2 + q
                            dy, dx = d // 3 - 1, d % 3 - 1
                            if dx == 0:
                                img_ap = imf[t][:, 66 + 64 * dy:66 + 64 * dy + SL]
                            elif dx == 1:
                                img_ap = ims[t][:, 66 + 64 * dy:66 + 64 * dy + SL]
                            else:
                                img_ap = ims[t][:, 64 + 64 * dy:64 + 64 * dy + SL]
                            pr = warp.tile([128, 512], BF, tag=f"pr{d}", bufs=2,
                                           name=f"pr{d}")
                            nc.vector.tensor_tensor(pr[:], img_ap,
                                                    psMb[:, q * 512:(q + 1) * 512],
                                                    AluOp.mult)
                            prods.append(pr)
                    # tree sum of 9 products, mostly DVE with GpSimd assist
                    s01 = warp.tile([128, 512], BF, tag="s01", bufs=2, name="s01")
                    nc.vector.tensor_tensor(s01[:], prods[0][:], prods[1][:], AluOp.add)
                    s23 = warp.tile([128, 512], BF, tag="s23", bufs=2, name="s23")
                    nc.vector.tensor_tensor(s23[:], prods[2][:], prods[3][:], AluOp.add)
                    s45 = warp.tile([128, 512], BF, tag="s45", bufs=2, name="s45")
                    nc.vector.tensor_tensor(s45[:], prods[4][:], prods[5][:], AluOp.add)
                    s67 = warp.tile([128, 512], BF, tag="s67", bufs=2, name="s67")
                    nc.gpsimd.tensor_tensor(s67[:], prods[6][:], prods[7][:], AluOp.add)
                    s03 = warp.tile([128, 512], BF, tag="s03", bufs=2, name="s03")
                    nc.vector.tensor_tensor(s03[:], s01[:], s23[:], AluOp.add)
                    s47 = warp.tile([128, 512], BF, tag="s47", bufs=2, name="s47")
                    nc.vector.tensor_tensor(s47[:], s45[:], s67[:], AluOp.add)
                    s07 = warp.tile([128, 512], BF, tag="s07", bufs=2, name="s07")
                    nc.vector.tensor_tensor(s07[:], s03[:], s47[:], AluOp.add)
                    out_t = warp.tile([128, 512], F32, tag="out", bufs=2, name="out_t")
                    nc.vector.tensor_tensor(out_t[:], s07[:], prods[8][:], AluOp.add)
                    nc.sync.dma_start(out_part[t, :, :], out_t[:])
            _late.close()

    nc.compile()
    return nc


def _prep_inputs(gar_feat, cond_feat, mask, W1, b1, W2, b2, W3, b3, W4, b4):
    """Host-side prep: returns list of 8 per-core input dicts."""
    gar = np.asarray(gar_feat, np.float32)[0]      # [256, 64, 64]
    cond = np.asarray(cond_feat, np.float32)[0]
    maskf = np.asarray(mask, np.float32)[0]        # [G, 256]

    inp = np.concatenate([gar, cond], axis=0)      # [512, 64, 64]
    inp_pad = np.zeros((C_IN, PADW, PADW), np.float32)
    inp_pad[:, 1:-1, 1:-1] = inp
    inp_chunks = inp_pad.reshape(4, 128, NPAD)
    x_np = np.zeros((2, 128, 2, XPL), np.float32)
    for pr in range(2):
        for ko in range(2):
            x_np[pr, :, ko, FPAD:FPAD + NPAD] = inp_chunks[pr * 2 + ko]
    x_np = np.clip(x_np, -240, 240).astype(FP8)

    # per-slice pixel coords (partition p = s*12 + k*2 + h; rows identical)
    # px_global = g*512 + h*256 + f
    def coords(g):
        pj = np.zeros((2, 256), np.float32)
        pi = np.zeros((2, 256), np.float32)
        for h in range(2):
            gpx = g * 512 + h * 256 + np.arange(256)
            pi[h] = gpx // W
            pj[h] = gpx % W
        i96 = np.tile(pi, (48, 1)).astype(np.float32)
        j96 = np.tile(pj, (48, 1)).astype(np.float32)
        return i96, j96

    sel16 = np.zeros((96, 16), np.float32)
    selbc = np.zeros((96, 96), np.float32)
    for p in range(96):
        s, rem = divmod(p, 12)
        k, h = divmod(rem, 2)
        sel16[p, s * 2 + h] = 1.0
        for s2 in range(8):
            selbc[s2 * 12 + rem, p] = 1.0
    sel16 = sel16.astype(BF16)
    selbc = selbc.astype(BF16)
    maskT = maskf.astype(BF16)                     # [8, 256]

    # flat gar image with wide guard, plus shift-by-one copy (for odd bases)
    gar_flat = gar.reshape(2, 128, HW)
    gpad = np.zeros((2, 128, 66 + HW + 67), np.float32)
    gpad[:, :, 66:66 + HW] = gar_flat

    per_core = []
    for g in range(G):
        w1g = np.asarray(W1[g], np.float32)   # [128, 512, 3, 3]
        w2g = np.asarray(W2[g], np.float32)   # [64, 128, 3, 3]
        w3g = np.asarray(W3[g], np.float32)   # [32, 64, 3, 3]
        w4g = np.asarray(W4[g], np.float32)[PERM4]   # [18, 32, 3, 3] permuted
        b4g = np.asarray(b4[g], np.float32)[PERM4]

        w1T = np.zeros((128, 36, 128), np.float32)
        for t in range(9):
            di, dj = t // 3, t % 3
            for pr in range(2):
                for ko in range(2):
                    idx = (t * 2 + pr) * 2 + ko
                    cin = (pr * 2 + ko) * 128
                    w1T[:, idx, :] = w1g[:, cin:cin + 128, di, dj].T * S1
        w1T = np.clip(w1T, -240, 240).astype(FP8)

        w2T = np.zeros((128, 9, 64), np.float32)
        for dj in range(3):
            for ko in range(2):  # taps (di=0,1) as DoubleRow pairs
                w2T[:, 2 * dj + ko, :] = w2g[:, :, ko, dj].T * S2
            w2T[:, 6 + dj, :] = w2g[:, :, 2, dj].T * S2
        w2T = np.clip(w2T, -240, 240).astype(FP8)

        w3T = np.zeros((128, 9, 32), np.float32)
        w4T = np.zeros((128, 9, 18), np.float32)
        for t in range(9):
            di, dj = t // 3, t % 3
            for bnd in range(2):
                w3T[bnd * 64:(bnd + 1) * 64, t, :] = w3g[:, :, di, dj].T
                # conv4 contraction is padded to 64 rows; rows 32:64 / 96:128
                # stay zero so the padding contributes nothing.
                w4T[bnd * 64:bnd * 64 + 32, t, :] = w4g[:, :, di, dj].T

        # per-core warp windows: global pixels [g*512-66, g*512+512+66)
        base = g * SL
        imgf = gpad[:, :, base:base + IMG_W]                  # offset -66
        imgs = gpad[:, :, base + 1:base + 1 + IMG_W]          # shift +1

        i96, j96 = coords(g)
        jsc = np.concatenate([j96 / 63.0 - 0.5, i96 / 63.0 - 0.5],
                             axis=1).astype(np.float32)
        lc = np.concatenate([np.maximum(-j96, -1.0), np.maximum(-i96, -1.0)],
                            axis=1).astype(np.float32)
        hc = np.concatenate([np.minimum(62.999 - j96, 1.0),
                             np.minimum(62.999 - i96, 1.0)],
                            axis=1).astype(np.float32)

        per_core.append({
            "x_dr": x_np,
            "img_f": np.ascontiguousarray(imgf).astype(BF16),
            "img_s": np.ascontiguousarray(imgs).astype(BF16),
            "w1d": w1T,
            "w2d": w2T,
            "w3d": w3T.astype(BF16),
            "w4d": w4T.astype(BF16),
            "b1d": np.asarray(b1[g], np.float32).reshape(128, 1),
            "b2d": np.asarray(b2[g], np.float32).reshape(64, 1),
            "b3d": np.asarray(b3[g], np.float32).reshape(32, 1),
            "b4d": (b4g * 64.0).reshape(18, 1),
            "jscd": jsc, "lcd": lc, "hcd": hc,
            "sel16d": sel16, "selbcd": selbc, "maskd": maskT,
        })
    return per_core


def _get_nc():
    if "nc" not in _CACHE:
        _CACHE["nc"] = _build()
    return _CACHE["nc"]


def run_cores(inputs, trace=False):
    nc = _get_nc()
    in_maps = _prep_inputs(**inputs)
    res = run_bass_kernel_spmd(nc, in_maps, core_ids=list(range(G)), trace=trace)
    return res


def kernel(**inputs) -> np.ndarray:
    res = run_cores(inputs, trace=False)
    out = np.zeros((C_FEAT, HW), np.float32)
    for g, r in enumerate(res.results):
        out[:, g * SL:(g + 1) * SL] = r["out_part"].reshape(C_FEAT, SL)
    return out.reshape(1, C_FEAT, H, W)
